# revision 26
# baseline (speedup 1.0000x reference)
"""HMM window log-likelihood on 8 NeuronCores (data-parallel over batch).

Math: reference computes, per batch column b,
    y[b] = exp(logsumexp_i x_T[b,i]),  x via log-space forward recursion.
Equivalently in linear space with row-normalized transition matrices
W_t = exp(w[t-1]) / rowsum, emission table L = softmax(distros, axis=1):
    y[b] = 1^T diag(em_T) W_T ... diag(em_1) W_1 em_0
Evaluated as a BACKWARD recursion:
    c_t = em_t . beta_t;  beta_{t-1} = W_t^T c_t;  colsum = 1^T c_0
with per-step rescale factors g_t (host f64, from column 0) folded into the
host-precomputed scaled emissions emsc[i,t,b] = L[i,bin(b,t)] * rsg[i,t].

Device structure (per core, BC=512 batch cols):
 - The 255-step serial chain is split into 6 SEGMENTS run as independent
   chains; each lower segment starts K steps early from beta=1 ("burn-in").
   The HMM forgets initial direction geometrically, so after K steps the
   burn-in beta is proportional to the true beta; the unknown per-column
   factor cancels exactly via junction sums J = 1^T c computed by BOTH
   chains at the junction step:  lnY += log(J_upper) - log(J_burnin).
 - Chains run in 3 lockstep PAIRS so each per-step op is one wide
   [128, 1024] instruction (2 chains x 512 cols).
 - Emissions arrive pre-scaled from host via DMA (bf16, SBUF) - no scalar
   activation and no K=10 emission matmuls (which bottlenecked v1).
 - Per-step c = em . beta with beta f32 in PSUM; lanes:
     pair 0 (direct): DVE tensor_mul(c, beta_psum_f32, em_bf16)  (1x mode)
     pairs 1,2 (cast): ScalarE casts beta f32->bf16, then DVE runs the
     multiply all-bf16/SBUF which triggers the DVE 2x_1P perf mode.
Device returns colsum + junction sums; host combines in f64:
lnY ~ -584.6 so y underflows f32 to 0.0, matching the reference exactly.
"""
import sys, os
for p in ("/opt/trn_rl_repo",):
    if p not in sys.path:
        sys.path.insert(0, p)
import numpy as np
import ml_dtypes

from concourse import bass, bacc, mybir
from concourse.tile import TileContext
from concourse.bass_utils import run_bass_kernel_spmd

W, L, B, NB = 128, 256, 4096, 10
NCORES = 8
BC = B // NCORES          # 512 batch cols per core
K_BURN = 8                # burn-in steps per junction
TBLK = 7                  # em streaming block (steps per DMA)
GP_MOD, GP_CNT = 7, 0     # cast-pair steps with i%GP_MOD<GP_CNT go to GpSimd

# Segment boundaries: 6 chains in 3 pairs, lockstep within a pair.
# chain s covers real t in [MLO[s], MHI[s]); burn-in K above MHI (top chain
# none).  Pair lanes: pair 2 = "direct" (DVE from PSUM), pairs 0,1 = "cast"
# (ACT f32->bf16 + DVE 2x / GpSimd).  Direct pair gets the top segments.
M_BOUND = [0, 34, 68, 102, 136, 192, 256]   # m0..m6
PAIRS = [(1, 2), (3, 4), (5, 6)]            # chain ids (1-based)
PAIR_LANE = ["cast", "cast", "direct"]
CHAIN_THI = {s: (M_BOUND[s] + K_BURN - 1 if s < 6 else 255) for s in range(1, 7)}
CHAIN_TLO = {s: M_BOUND[s - 1] for s in range(1, 7)}
PAIR_STEPS = [CHAIN_THI[a] - CHAIN_TLO[a] + 1 for a, _ in PAIRS]
for pi, (a, b) in enumerate(PAIRS):
    assert CHAIN_THI[a] - CHAIN_TLO[a] == CHAIN_THI[b] - CHAIN_TLO[b], (pi, a, b)

LAST_LNY = None           # debug: device-derived lnY per batch col
LAST_RESULTS = None       # debug: raw BassKernelResults

_CACHED = None


def _build_nc():
    nc = bacc.Bacc("TRN2", target_bir_lowering=False, debug=False,
                   num_devices=NCORES)
    bf16, f32 = mybir.dt.bfloat16, mybir.dt.float32
    Copy = mybir.ActivationFunctionType.Copy

    wt = nc.dram_tensor("wt", [W, L - 1, W], bf16, kind="ExternalInput")
    ems = [nc.dram_tensor(f"em{pi}", [W, PAIR_STEPS[pi], 2 * BC], bf16,
                          kind="ExternalInput") for pi in range(3)]
    ones = nc.dram_tensor("ones", [W, 1], bf16, kind="ExternalInput")
    juncs = nc.dram_tensor("juncs", [12, BC], f32, kind="ExternalOutput")

    with TileContext(nc) as tc:
        with tc.sbuf_pool(name="sb", bufs=2) as sb, \
                tc.psum_pool(name="ps", bufs=2) as ps:
            ones_sb = sb.tile([W, 1], bf16, bufs=1)
            nc.sync.dma_start(ones_sb, ones.ap())

            # transition matrices resident; per-chain backward chunks so
            # every chain's first weights land early.  Emitted AFTER the
            # first em blocks (below) so chain startup isn't stuck behind
            # 8 MB of weight traffic.
            em_blocks = {}
            FBLK = 2   # tiny first block so chains start ~immediately

            def blk_of(i):
                return 0 if i < FBLK else 1 + (i - FBLK) // TBLK

            def load_block(pi, bi):
                eblk = sb.tile([W, TBLK, 2 * BC], bf16,
                               tag=f"em{pi}", bufs=2)
                i0 = 0 if bi == 0 else FBLK + (bi - 1) * TBLK
                i1 = min((FBLK if bi == 0 else i0 + TBLK), PAIR_STEPS[pi])
                nc.sync.dma_start(eblk[:, :i1 - i0, :],
                                  ems[pi].ap()[:, i0:i1, :])
                em_blocks[(pi, bi)] = (eblk, i0)

            for _pi in range(3):
                load_block(_pi, 0)   # chain startup data first in DMA order

            wt_sb = sb.tile([W, L - 1, W], bf16, bufs=1)
            WCH = 16
            chunks = []   # (order_key, aligned_block_index)
            for s in range(1, 7):
                lo = max(CHAIN_TLO[s] - 1, 0)
                hi = CHAIN_THI[s] - 1          # MM at step t uses wt[t-1]
                blks = list(range(hi // WCH, lo // WCH - 1, -1))
                for oi, bi in enumerate(blks):
                    chunks.append((oi, bi))
            chunks.sort()
            seen = set()
            for _, bi in chunks:
                if bi in seen:
                    continue
                seen.add(bi)
                t0 = bi * WCH
                cnt = min(WCH, L - 1 - t0)
                nc.sync.dma_start(wt_sb[:, t0:t0 + cnt, :],
                                  wt.ap()[:, t0:t0 + cnt, :])

            nsteps = max(PAIR_STEPS)
            beta = [None, None, None]       # per-pair PSUM [W, 2*BC] f32
            csb = [None, None, None]        # per-pair SBUF c [W, 2*BC] bf16
            junc_row = [0]

            def em_slice(pi, i):
                bi = blk_of(i)
                if (pi, bi) not in em_blocks:
                    load_block(pi, bi)
                eblk, i0 = em_blocks[(pi, bi)]
                return eblk[:, i - i0, :]

            def junction_sum(c_half):
                # J = 1^T c -> PSUM [1, BC] -> SBUF -> DMA'd out immediately
                j_ps = ps.tile([1, BC], f32, tag="junc", bufs=1)
                nc.tensor.matmul(j_ps, ones_sb, c_half, start=True, stop=True)
                j_sb = sb.tile([1, BC], f32, tag="jsb", bufs=2)
                nc.vector.tensor_copy(j_sb, j_ps)
                row = junc_row[0]
                nc.sync.dma_start(juncs.ap()[row:row + 1, :], j_sb)
                junc_row[0] += 1

            # dummy matmuls into the spare PSUM bank keep the PE HAM
            # clock-gate warm (idle-gapped MMs otherwise run at 1.2 GHz)
            warm_ps = ps.tile([1, BC], f32, tag="warm", bufs=1)

            def warm_mm():
                nc.tensor.matmul(warm_ps, ones_sb, wt_sb[:, 0:4, :],
                                 start=True, stop=True)

            for i in range(nsteps):
                for pi, (sa, sb_id) in enumerate(PAIRS):
                    if i >= PAIR_STEPS[pi]:
                        continue
                    lane = PAIR_LANE[pi]
                    em_i = em_slice(pi, i)
                    if i == 0:
                        c = em_i           # beta = 1 at chain start
                    else:
                        c = sb.tile([W, 2 * BC], bf16, tag=f"c{pi}", bufs=2)
                        if lane == "direct":
                            nc.vector.tensor_mul(c, beta[pi], em_i)
                        else:
                            bsb = sb.tile([W, 2 * BC], bf16, tag=f"bs{pi}",
                                          bufs=2)
                            nc.scalar.activation(bsb, beta[pi], Copy)
                            if i % GP_MOD < GP_CNT:
                                nc.gpsimd.tensor_mul(c, bsb, em_i)
                            else:
                                nc.vector.tensor_mul(c, bsb, em_i)
                    csb[pi] = c
                    if any(CHAIN_THI[s] - i > CHAIN_TLO[s]
                           for s in (sa, sb_id)):
                        bnew = ps.tile([W, 2 * BC], f32,
                                       tag=f"b{pi}", bufs=1)
                        beta[pi] = bnew
                    for h, s in enumerate((sa, sb_id)):
                        t = CHAIN_THI[s] - i
                        c_half = c[:, h * BC:(h + 1) * BC]
                        if t == CHAIN_TLO[s]:
                            junction_sum(c_half)     # last c of this chain
                        elif t == M_BOUND[s] and s < 6:
                            junction_sum(c_half)     # burn-in end J
                        if t > CHAIN_TLO[s]:
                            nc.tensor.matmul(
                                beta[pi][:, h * BC:(h + 1) * BC],
                                wt_sb[:, t - 1, :], c_half,
                                start=True, stop=True)
                    warm_mm()

            assert junc_row[0] == 11, junc_row[0]
    nc.compile()
    return nc


def _host_prep(data, input_distros, dense_layer_weights):
    f64 = np.float64
    we = np.exp(dense_layer_weights.astype(f64))           # (255,W,W)
    rowsum = we.sum(axis=2)                                # (255,W)
    recip = 1.0 / rowsum
    d = input_distros.astype(f64)
    d = d - d.max(axis=1, keepdims=True)
    e = np.exp(d)
    Ll = e / e.sum(axis=1, keepdims=True)                  # (W,NB)
    bins = np.minimum(NB - 1, np.floor(
        data / np.float32(0.1)).astype(np.int32))          # (B,L)

    # column-0 f64 backward pass -> per-step rescale g_t, offset Cacc
    beta = np.ones(W, dtype=f64)
    Cacc = 0.0
    g = np.ones(L, dtype=f64)
    for t in range(L - 1, 0, -1):
        c = Ll[np.arange(W), bins[0, t]] * beta * recip[t - 1]
        tmp = we[t - 1].T @ c
        f = tmp.max()
        g[t] = 1.0 / f
        Cacc += np.log(f)
        beta = tmp * g[t]

    rsg = np.ones((W, L), dtype=f64)
    rsg[:, 1:] = recip.T * g[None, 1:]
    Lsc = (Ll[:, None, :] * rsg[:, :, None]).astype(np.float32)  # (W,L,NB)

    wt = np.ascontiguousarray(
        we.transpose(1, 0, 2)).astype(ml_dtypes.bfloat16)  # (W,255,W)
    ones_v = np.ones((W, 1), dtype=ml_dtypes.bfloat16)
    return wt, Lsc, bins, ones_v, Cacc


def _build_em_pair(Lsc, bins_core, pi):
    """em stream for pair pi: [W, steps, 2*BC] bf16.
    em[:, i, h*BC:(h+1)*BC] = Lsc[:, t, bins[:, t]] for chain h at step i."""
    steps = PAIR_STEPS[pi]
    out = np.empty((W, steps, 2 * BC), dtype=ml_dtypes.bfloat16)
    for h, s in enumerate(PAIRS[pi]):
        ts = CHAIN_THI[s] - np.arange(steps)                # (steps,)
        # gather: for each step i: Lsc[:, ts[i], bins_core[:, ts[i]]]
        bsel = bins_core[:, ts]                             # (BC, steps)
        blk = Lsc[:, ts[:, None], bsel.T]                   # (W, steps, BC)
        out[:, :, h * BC:(h + 1) * BC] = blk.astype(ml_dtypes.bfloat16)
    return out


def kernel(data, input_distros, dense_layer_weights):
    global LAST_LNY, LAST_RESULTS, _CACHED
    wt, Lsc, bins, ones_v, Cacc = _host_prep(
        np.asarray(data), np.asarray(input_distros),
        np.asarray(dense_layer_weights))

    if _CACHED is None:
        _CACHED = _build_nc()
    nc = _CACHED

    in_maps = []
    for cid in range(NCORES):
        bins_core = bins[cid * BC:(cid + 1) * BC, :]        # (BC, L)
        m = {"wt": wt, "ones": ones_v}
        for pi in range(3):
            m[f"em{pi}"] = _build_em_pair(Lsc, bins_core, pi)
        in_maps.append(m)

    res = run_bass_kernel_spmd(
        nc, in_maps, core_ids=list(range(NCORES)),
        trace=bool(int(os.environ.get("KERNEL_TRACE", "0"))))
    LAST_RESULTS = res

    # rows: emitted in program order per junction_sum() calls:
    #  pair0 chains (1,2) then pair1 (3,4) then pair2 (5,6), interleaved by
    #  step; mapping below reconstructs which row is which junction sum.
    rows = {}
    ri = 0
    for i in range(max(PAIR_STEPS)):
        for pi, (sa, sb_id) in enumerate(PAIRS):
            if i >= PAIR_STEPS[pi]:
                continue
            for s in (sa, sb_id):
                t = CHAIN_THI[s] - i
                if t == CHAIN_TLO[s]:
                    rows[("lo", s)] = ri; ri += 1
                elif t == M_BOUND[s] and s < 6:
                    rows[("burn", s)] = ri; ri += 1
    assert ri == 11, ri

    lnY = np.zeros(B, dtype=np.float64)
    for cid in range(NCORES):
        jr = LAST_RESULTS.results[cid]["juncs"].astype(np.float64)
        acc = np.log(jr[rows[("lo", 1)]])                   # colsum at t=0
        for s in range(1, 6):
            acc += np.log(jr[rows[("lo", s + 1)]])          # J_upper
            acc -= np.log(jr[rows[("burn", s)]])            # J_burn-in
        lnY[cid * BC:(cid + 1) * BC] = acc + Cacc
    LAST_LNY = lnY
    y = np.exp(lnY).astype(np.float32).reshape(B, 1)
    return y


# revision 28
# speedup vs baseline: 1.1899x; 1.1899x over previous
"""HMM window log-likelihood on 8 NeuronCores (data-parallel over batch).

Math: reference computes, per batch column b,
    y[b] = exp(logsumexp_i x_T[b,i]),  x via log-space forward recursion.
Equivalently in linear space with row-normalized transition matrices
W_t = exp(w[t-1]) / rowsum, emission table L = softmax(distros, axis=1):
    y[b] = 1^T diag(em_T) W_T ... diag(em_1) W_1 em_0
Evaluated as a BACKWARD recursion:
    c_t = em_t . beta_t;  beta_{t-1} = W_t^T c_t;  colsum = 1^T c_0
with per-step rescale factors g_t (host f64, from column 0) folded into the
host-precomputed scaled emissions emsc[i,t,b] = L[i,bin(b,t)] * rsg[i,t].

Device structure (per core, BC=512 batch cols):
 - The 255-step serial chain is split into 6 SEGMENTS run as independent
   chains; each lower segment starts K steps early from beta=1 ("burn-in").
   The HMM forgets initial direction geometrically, so after K steps the
   burn-in beta is proportional to the true beta; the unknown per-column
   factor cancels exactly via junction sums J = 1^T c computed by BOTH
   chains at the junction step:  lnY += log(J_upper) - log(J_burnin).
 - Chains run in 3 lockstep PAIRS so each per-step op is one wide
   [128, 1024] instruction (2 chains x 512 cols).
 - Emissions arrive pre-scaled from host via DMA (bf16, SBUF) - no scalar
   activation and no K=10 emission matmuls (which bottlenecked v1).
 - Per-step c = em . beta with beta f32 in PSUM; lanes:
     pair 0 (direct): DVE tensor_mul(c, beta_psum_f32, em_bf16)  (1x mode)
     pairs 1,2 (cast): ScalarE casts beta f32->bf16, then DVE runs the
     multiply all-bf16/SBUF which triggers the DVE 2x_1P perf mode.
Device returns colsum + junction sums; host combines in f64:
lnY ~ -584.6 so y underflows f32 to 0.0, matching the reference exactly.
"""
import sys, os
for p in ("/opt/trn_rl_repo",):
    if p not in sys.path:
        sys.path.insert(0, p)
import numpy as np
import ml_dtypes

from concourse import bass, bacc, mybir
from concourse.tile import TileContext
from concourse.bass_utils import run_bass_kernel_spmd

W, L, B, NB = 128, 256, 4096, 10
NCORES = 8
BC = B // NCORES          # 512 batch cols per core
K_BURN = 8                # burn-in steps per junction
TBLK = 7                  # em streaming block (steps per DMA)
GP_MOD, GP_CNT = 7, 0     # cast-pair steps with i%GP_MOD<GP_CNT go to GpSimd

# Segment boundaries: 6 chains in 3 pairs, lockstep within a pair.
# chain s covers real t in [MLO[s], MHI[s]); burn-in K above MHI (top chain
# none).  Pair lanes: pair 2 = "direct" (DVE from PSUM), pairs 0,1 = "cast"
# (ACT f32->bf16 + DVE 2x / GpSimd).  Direct pair gets the top segments.
M_BOUND = [0, 34, 68, 102, 136, 192, 256]   # m0..m6
PAIRS = [(1, 2), (3, 4), (5, 6)]            # chain ids (1-based)
PAIR_LANE = ["cast", "cast", "direct"]
CHAIN_THI = {s: (M_BOUND[s] + K_BURN - 1 if s < 6 else 255) for s in range(1, 7)}
CHAIN_TLO = {s: M_BOUND[s - 1] for s in range(1, 7)}
PAIR_STEPS = [CHAIN_THI[a] - CHAIN_TLO[a] + 1 for a, _ in PAIRS]
for pi, (a, b) in enumerate(PAIRS):
    assert CHAIN_THI[a] - CHAIN_TLO[a] == CHAIN_THI[b] - CHAIN_TLO[b], (pi, a, b)

LAST_LNY = None           # debug: device-derived lnY per batch col
LAST_RESULTS = None       # debug: raw BassKernelResults

_CACHED = None


def _build_nc():
    nc = bacc.Bacc("TRN2", target_bir_lowering=False, debug=False,
                   num_devices=NCORES)
    bf16, f32 = mybir.dt.bfloat16, mybir.dt.float32
    Copy = mybir.ActivationFunctionType.Copy

    wt = nc.dram_tensor("wt", [W, L - 1, W], bf16, kind="ExternalInput")
    ems = [nc.dram_tensor(f"em{pi}", [W, PAIR_STEPS[pi], 2 * BC], bf16,
                          kind="ExternalInput") for pi in range(3)]
    ones = nc.dram_tensor("ones", [W, 1], bf16, kind="ExternalInput")
    juncs = nc.dram_tensor("juncs", [12, BC], f32, kind="ExternalOutput")

    with TileContext(nc) as tc:
        with tc.sbuf_pool(name="sb", bufs=2) as sb, \
                tc.psum_pool(name="ps", bufs=2) as ps:
            ones_sb = sb.tile([W, 1], bf16, bufs=1)
            nc.sync.dma_start(ones_sb, ones.ap())

            # transition matrices resident; per-chain backward chunks so
            # every chain's first weights land early.  Emitted AFTER the
            # first em blocks (below) so chain startup isn't stuck behind
            # 8 MB of weight traffic.
            em_blocks = {}
            FBLK = 2   # tiny first block so chains start ~immediately

            def blk_of(i):
                return 0 if i < FBLK else 1 + (i - FBLK) // TBLK

            def load_block(pi, bi):
                eblk = sb.tile([W, TBLK, 2 * BC], bf16,
                               tag=f"em{pi}", bufs=2)
                i0 = 0 if bi == 0 else FBLK + (bi - 1) * TBLK
                i1 = min((FBLK if bi == 0 else i0 + TBLK), PAIR_STEPS[pi])
                nc.sync.dma_start(eblk[:, :i1 - i0, :],
                                  ems[pi].ap()[:, i0:i1, :])
                em_blocks[(pi, bi)] = (eblk, i0)

            for _pi in range(3):
                load_block(_pi, 0)   # chain startup data first in DMA order

            wt_sb = sb.tile([W, L - 1, W], bf16, bufs=1)
            WCH = 16
            chunks = []   # (order_key, aligned_block_index)
            for s in range(1, 7):
                lo = max(CHAIN_TLO[s] - 1, 0)
                hi = CHAIN_THI[s] - 1          # MM at step t uses wt[t-1]
                blks = list(range(hi // WCH, lo // WCH - 1, -1))
                for oi, bi in enumerate(blks):
                    chunks.append((oi, bi))
            chunks.sort()
            seen = set()
            for _, bi in chunks:
                if bi in seen:
                    continue
                seen.add(bi)
                t0 = bi * WCH
                cnt = min(WCH, L - 1 - t0)
                nc.sync.dma_start(wt_sb[:, t0:t0 + cnt, :],
                                  wt.ap()[:, t0:t0 + cnt, :])

            nsteps = max(PAIR_STEPS)
            beta = [None, None, None]       # per-pair PSUM [W, 2*BC] f32
            csb = [None, None, None]        # per-pair SBUF c [W, 2*BC] bf16
            junc_row = [0]

            def em_slice(pi, i):
                bi = blk_of(i)
                if (pi, bi) not in em_blocks:
                    load_block(pi, bi)
                eblk, i0 = em_blocks[(pi, bi)]
                return eblk[:, i - i0, :]

            def junction_sum(c_half):
                # J = 1^T c -> PSUM [1, BC] -> SBUF -> DMA'd out immediately
                j_ps = ps.tile([1, BC], f32, tag="junc", bufs=1)
                nc.tensor.matmul(j_ps, ones_sb, c_half, start=True, stop=True)
                j_sb = sb.tile([1, BC], f32, tag="jsb", bufs=2)
                nc.vector.tensor_copy(j_sb, j_ps)
                row = junc_row[0]
                nc.sync.dma_start(juncs.ap()[row:row + 1, :], j_sb)
                junc_row[0] += 1

            for i in range(nsteps):
                for pi, (sa, sb_id) in enumerate(PAIRS):
                    if i >= PAIR_STEPS[pi]:
                        continue
                    lane = PAIR_LANE[pi]
                    em_i = em_slice(pi, i)
                    if i == 0:
                        c = em_i           # beta = 1 at chain start
                    else:
                        c = sb.tile([W, 2 * BC], bf16, tag=f"c{pi}", bufs=2)
                        if lane == "direct":
                            nc.vector.tensor_mul(c, beta[pi], em_i)
                        else:
                            bsb = sb.tile([W, 2 * BC], bf16, tag=f"bs{pi}",
                                          bufs=2)
                            nc.scalar.activation(bsb, beta[pi], Copy)
                            if i % GP_MOD < GP_CNT:
                                nc.gpsimd.tensor_mul(c, bsb, em_i)
                            else:
                                nc.vector.tensor_mul(c, bsb, em_i)
                    csb[pi] = c
                    if any(CHAIN_THI[s] - i > CHAIN_TLO[s]
                           for s in (sa, sb_id)):
                        bnew = ps.tile([W, 2 * BC], f32,
                                       tag=f"b{pi}", bufs=1)
                        beta[pi] = bnew
                    for h, s in enumerate((sa, sb_id)):
                        t = CHAIN_THI[s] - i
                        c_half = c[:, h * BC:(h + 1) * BC]
                        if t == CHAIN_TLO[s]:
                            junction_sum(c_half)     # last c of this chain
                        elif t == M_BOUND[s] and s < 6:
                            junction_sum(c_half)     # burn-in end J
                        if t > CHAIN_TLO[s]:
                            nc.tensor.matmul(
                                beta[pi][:, h * BC:(h + 1) * BC],
                                wt_sb[:, t - 1, :], c_half,
                                start=True, stop=True)

            assert junc_row[0] == 11, junc_row[0]
    nc.compile()
    return nc


def _host_prep(data, input_distros, dense_layer_weights):
    f64 = np.float64
    we = np.exp(dense_layer_weights.astype(f64))           # (255,W,W)
    rowsum = we.sum(axis=2)                                # (255,W)
    recip = 1.0 / rowsum
    d = input_distros.astype(f64)
    d = d - d.max(axis=1, keepdims=True)
    e = np.exp(d)
    Ll = e / e.sum(axis=1, keepdims=True)                  # (W,NB)
    bins = np.minimum(NB - 1, np.floor(
        data / np.float32(0.1)).astype(np.int32))          # (B,L)

    # column-0 f64 backward pass -> per-step rescale g_t, offset Cacc
    beta = np.ones(W, dtype=f64)
    Cacc = 0.0
    g = np.ones(L, dtype=f64)
    for t in range(L - 1, 0, -1):
        c = Ll[np.arange(W), bins[0, t]] * beta * recip[t - 1]
        tmp = we[t - 1].T @ c
        f = tmp.max()
        g[t] = 1.0 / f
        Cacc += np.log(f)
        beta = tmp * g[t]

    rsg = np.ones((W, L), dtype=f64)
    rsg[:, 1:] = recip.T * g[None, 1:]
    Lsc = (Ll[:, None, :] * rsg[:, :, None]).astype(np.float32)  # (W,L,NB)

    wt = np.ascontiguousarray(
        we.transpose(1, 0, 2)).astype(ml_dtypes.bfloat16)  # (W,255,W)
    ones_v = np.ones((W, 1), dtype=ml_dtypes.bfloat16)
    return wt, Lsc, bins, ones_v, Cacc


def _build_em_pair(Lsc, bins_core, pi):
    """em stream for pair pi: [W, steps, 2*BC] bf16.
    em[:, i, h*BC:(h+1)*BC] = Lsc[:, t, bins[:, t]] for chain h at step i."""
    steps = PAIR_STEPS[pi]
    out = np.empty((W, steps, 2 * BC), dtype=ml_dtypes.bfloat16)
    for h, s in enumerate(PAIRS[pi]):
        ts = CHAIN_THI[s] - np.arange(steps)                # (steps,)
        # gather: for each step i: Lsc[:, ts[i], bins_core[:, ts[i]]]
        bsel = bins_core[:, ts]                             # (BC, steps)
        blk = Lsc[:, ts[:, None], bsel.T]                   # (W, steps, BC)
        out[:, :, h * BC:(h + 1) * BC] = blk.astype(ml_dtypes.bfloat16)
    return out


def kernel(data, input_distros, dense_layer_weights):
    global LAST_LNY, LAST_RESULTS, _CACHED
    wt, Lsc, bins, ones_v, Cacc = _host_prep(
        np.asarray(data), np.asarray(input_distros),
        np.asarray(dense_layer_weights))

    if _CACHED is None:
        _CACHED = _build_nc()
    nc = _CACHED

    in_maps = []
    for cid in range(NCORES):
        bins_core = bins[cid * BC:(cid + 1) * BC, :]        # (BC, L)
        m = {"wt": wt, "ones": ones_v}
        for pi in range(3):
            m[f"em{pi}"] = _build_em_pair(Lsc, bins_core, pi)
        in_maps.append(m)

    res = run_bass_kernel_spmd(
        nc, in_maps, core_ids=list(range(NCORES)),
        trace=bool(int(os.environ.get("KERNEL_TRACE", "0"))))
    LAST_RESULTS = res

    # rows: emitted in program order per junction_sum() calls:
    #  pair0 chains (1,2) then pair1 (3,4) then pair2 (5,6), interleaved by
    #  step; mapping below reconstructs which row is which junction sum.
    rows = {}
    ri = 0
    for i in range(max(PAIR_STEPS)):
        for pi, (sa, sb_id) in enumerate(PAIRS):
            if i >= PAIR_STEPS[pi]:
                continue
            for s in (sa, sb_id):
                t = CHAIN_THI[s] - i
                if t == CHAIN_TLO[s]:
                    rows[("lo", s)] = ri; ri += 1
                elif t == M_BOUND[s] and s < 6:
                    rows[("burn", s)] = ri; ri += 1
    assert ri == 11, ri

    lnY = np.zeros(B, dtype=np.float64)
    for cid in range(NCORES):
        jr = LAST_RESULTS.results[cid]["juncs"].astype(np.float64)
        acc = np.log(jr[rows[("lo", 1)]])                   # colsum at t=0
        for s in range(1, 6):
            acc += np.log(jr[rows[("lo", s + 1)]])          # J_upper
            acc -= np.log(jr[rows[("burn", s)]])            # J_burn-in
        lnY[cid * BC:(cid + 1) * BC] = acc + Cacc
    LAST_LNY = lnY
    y = np.exp(lnY).astype(np.float32).reshape(B, 1)
    return y


# revision 29
# speedup vs baseline: 1.3184x; 1.1080x over previous
"""HMM window log-likelihood on 8 NeuronCores (data-parallel over batch).

Math: reference computes, per batch column b,
    y[b] = exp(logsumexp_i x_T[b,i]),  x via log-space forward recursion.
Equivalently in linear space with row-normalized transition matrices
W_t = exp(w[t-1]) / rowsum, emission table L = softmax(distros, axis=1):
    y[b] = 1^T diag(em_T) W_T ... diag(em_1) W_1 em_0
Evaluated as a BACKWARD recursion:
    c_t = em_t . beta_t;  beta_{t-1} = W_t^T c_t;  colsum = 1^T c_0
with per-step rescale factors g_t (host f64, from column 0) folded into the
host-precomputed scaled emissions emsc[i,t,b] = L[i,bin(b,t)] * rsg[i,t].

Device structure (per core, BC=512 batch cols):
 - The 255-step serial chain is split into 6 SEGMENTS run as independent
   chains; each lower segment starts K steps early from beta=1 ("burn-in").
   The HMM forgets initial direction geometrically, so after K steps the
   burn-in beta is proportional to the true beta; the unknown per-column
   factor cancels exactly via junction sums J = 1^T c computed by BOTH
   chains at the junction step:  lnY += log(J_upper) - log(J_burnin).
 - Chains run in 3 lockstep PAIRS so each per-step op is one wide
   [128, 1024] instruction (2 chains x 512 cols).
 - Emissions arrive pre-scaled from host via DMA (bf16, SBUF) - no scalar
   activation and no K=10 emission matmuls (which bottlenecked v1).
 - Per-step c = em . beta with beta f32 in PSUM; lanes:
     pair 0 (direct): DVE tensor_mul(c, beta_psum_f32, em_bf16)  (1x mode)
     pairs 1,2 (cast): ScalarE casts beta f32->bf16, then DVE runs the
     multiply all-bf16/SBUF which triggers the DVE 2x_1P perf mode.
Device returns colsum + junction sums; host combines in f64:
lnY ~ -584.6 so y underflows f32 to 0.0, matching the reference exactly.
"""
import sys, os
for p in ("/opt/trn_rl_repo",):
    if p not in sys.path:
        sys.path.insert(0, p)
import numpy as np
import ml_dtypes

from concourse import bass, bacc, mybir
from concourse.tile import TileContext
from concourse.bass_utils import run_bass_kernel_spmd

W, L, B, NB = 128, 256, 4096, 10
NCORES = 8
BC = B // NCORES          # 512 batch cols per core
K_BURN = 8                # burn-in steps per junction
TBLK = 7                  # em streaming block (steps per DMA)
GP_MOD, GP_CNT = 7, 0     # cast-pair steps with i%GP_MOD<GP_CNT go to GpSimd

# Segment boundaries: 6 chains in 3 pairs, lockstep within a pair.
# chain s covers real t in [MLO[s], MHI[s]); burn-in K above MHI (top chain
# none).  Pair lanes: pair 2 = "direct" (DVE from PSUM), pairs 0,1 = "cast"
# (ACT f32->bf16 + DVE 2x / GpSimd).  Direct pair gets the top segments.
M_BOUND = [0, 38, 76, 114, 152, 200, 256]   # m0..m6
PAIRS = [(1, 2), (3, 4), (5, 6)]            # chain ids (1-based)
PAIR_LANE = ["cast", "cast", "direct"]
CHAIN_THI = {s: (M_BOUND[s] + K_BURN - 1 if s < 6 else 255) for s in range(1, 7)}
CHAIN_TLO = {s: M_BOUND[s - 1] for s in range(1, 7)}
PAIR_STEPS = [CHAIN_THI[a] - CHAIN_TLO[a] + 1 for a, _ in PAIRS]
for pi, (a, b) in enumerate(PAIRS):
    assert CHAIN_THI[a] - CHAIN_TLO[a] == CHAIN_THI[b] - CHAIN_TLO[b], (pi, a, b)

LAST_LNY = None           # debug: device-derived lnY per batch col
LAST_RESULTS = None       # debug: raw BassKernelResults

_CACHED = None


def _build_nc():
    nc = bacc.Bacc("TRN2", target_bir_lowering=False, debug=False,
                   num_devices=NCORES)
    bf16, f32 = mybir.dt.bfloat16, mybir.dt.float32
    Copy = mybir.ActivationFunctionType.Copy

    wt = nc.dram_tensor("wt", [W, L - 1, W], bf16, kind="ExternalInput")
    ems = [nc.dram_tensor(f"em{pi}", [W, PAIR_STEPS[pi], 2 * BC], bf16,
                          kind="ExternalInput") for pi in range(3)]
    ones = nc.dram_tensor("ones", [W, 1], bf16, kind="ExternalInput")
    juncs = nc.dram_tensor("juncs", [12, BC], f32, kind="ExternalOutput")

    with TileContext(nc) as tc:
        with tc.sbuf_pool(name="sb", bufs=2) as sb, \
                tc.psum_pool(name="ps", bufs=2) as ps:
            ones_sb = sb.tile([W, 1], bf16, bufs=1)
            nc.sync.dma_start(ones_sb, ones.ap())

            # transition matrices resident; per-chain backward chunks so
            # every chain's first weights land early.  Emitted AFTER the
            # first em blocks (below) so chain startup isn't stuck behind
            # 8 MB of weight traffic.
            em_blocks = {}
            FBLK = 2   # tiny first block so chains start ~immediately

            def blk_of(i):
                return 0 if i < FBLK else 1 + (i - FBLK) // TBLK

            def load_block(pi, bi):
                eblk = sb.tile([W, TBLK, 2 * BC], bf16,
                               tag=f"em{pi}", bufs=2)
                i0 = 0 if bi == 0 else FBLK + (bi - 1) * TBLK
                i1 = min((FBLK if bi == 0 else i0 + TBLK), PAIR_STEPS[pi])
                nc.sync.dma_start(eblk[:, :i1 - i0, :],
                                  ems[pi].ap()[:, i0:i1, :])
                em_blocks[(pi, bi)] = (eblk, i0)

            for _pi in range(3):
                load_block(_pi, 0)   # chain startup data first in DMA order

            wt_sb = sb.tile([W, L - 1, W], bf16, bufs=1)
            WCH = 16
            chunks = []   # (order_key, aligned_block_index)
            for s in range(1, 7):
                lo = max(CHAIN_TLO[s] - 1, 0)
                hi = CHAIN_THI[s] - 1          # MM at step t uses wt[t-1]
                blks = list(range(hi // WCH, lo // WCH - 1, -1))
                for oi, bi in enumerate(blks):
                    chunks.append((oi, bi))
            chunks.sort()
            seen = set()
            for _, bi in chunks:
                if bi in seen:
                    continue
                seen.add(bi)
                t0 = bi * WCH
                cnt = min(WCH, L - 1 - t0)
                nc.sync.dma_start(wt_sb[:, t0:t0 + cnt, :],
                                  wt.ap()[:, t0:t0 + cnt, :])

            nsteps = max(PAIR_STEPS)
            beta = [None, None, None]       # per-pair PSUM [W, 2*BC] f32
            csb = [None, None, None]        # per-pair SBUF c [W, 2*BC] bf16
            junc_row = [0]

            def em_slice(pi, i):
                bi = blk_of(i)
                if (pi, bi) not in em_blocks:
                    load_block(pi, bi)
                eblk, i0 = em_blocks[(pi, bi)]
                return eblk[:, i - i0, :]

            def junction_sum(c_half):
                # J = 1^T c -> PSUM [1, BC] -> SBUF -> DMA'd out immediately
                j_ps = ps.tile([1, BC], f32, tag="junc", bufs=1)
                nc.tensor.matmul(j_ps, ones_sb, c_half, start=True, stop=True)
                j_sb = sb.tile([1, BC], f32, tag="jsb", bufs=2)
                nc.vector.tensor_copy(j_sb, j_ps)
                row = junc_row[0]
                nc.sync.dma_start(juncs.ap()[row:row + 1, :], j_sb)
                junc_row[0] += 1

            for i in range(nsteps):
                for pi, (sa, sb_id) in enumerate(PAIRS):
                    if i >= PAIR_STEPS[pi]:
                        continue
                    lane = PAIR_LANE[pi]
                    em_i = em_slice(pi, i)
                    if i == 0:
                        c = em_i           # beta = 1 at chain start
                    else:
                        c = sb.tile([W, 2 * BC], bf16, tag=f"c{pi}", bufs=2)
                        if lane == "direct":
                            nc.vector.tensor_mul(c, beta[pi], em_i)
                        else:
                            bsb = sb.tile([W, 2 * BC], bf16, tag=f"bs{pi}",
                                          bufs=2)
                            nc.scalar.activation(bsb, beta[pi], Copy)
                            if i % GP_MOD < GP_CNT:
                                nc.gpsimd.tensor_mul(c, bsb, em_i)
                            else:
                                nc.vector.tensor_mul(c, bsb, em_i)
                    csb[pi] = c
                    if any(CHAIN_THI[s] - i > CHAIN_TLO[s]
                           for s in (sa, sb_id)):
                        bnew = ps.tile([W, 2 * BC], f32,
                                       tag=f"b{pi}", bufs=1)
                        beta[pi] = bnew
                    for h, s in enumerate((sa, sb_id)):
                        t = CHAIN_THI[s] - i
                        c_half = c[:, h * BC:(h + 1) * BC]
                        if t == CHAIN_TLO[s]:
                            junction_sum(c_half)     # last c of this chain
                        elif t == M_BOUND[s] and s < 6:
                            junction_sum(c_half)     # burn-in end J
                        if t > CHAIN_TLO[s]:
                            nc.tensor.matmul(
                                beta[pi][:, h * BC:(h + 1) * BC],
                                wt_sb[:, t - 1, :], c_half,
                                start=True, stop=True)

            assert junc_row[0] == 11, junc_row[0]
    nc.compile()
    return nc


def _host_prep(data, input_distros, dense_layer_weights):
    f64 = np.float64
    we = np.exp(dense_layer_weights.astype(f64))           # (255,W,W)
    rowsum = we.sum(axis=2)                                # (255,W)
    recip = 1.0 / rowsum
    d = input_distros.astype(f64)
    d = d - d.max(axis=1, keepdims=True)
    e = np.exp(d)
    Ll = e / e.sum(axis=1, keepdims=True)                  # (W,NB)
    bins = np.minimum(NB - 1, np.floor(
        data / np.float32(0.1)).astype(np.int32))          # (B,L)

    # column-0 f64 backward pass -> per-step rescale g_t, offset Cacc
    beta = np.ones(W, dtype=f64)
    Cacc = 0.0
    g = np.ones(L, dtype=f64)
    for t in range(L - 1, 0, -1):
        c = Ll[np.arange(W), bins[0, t]] * beta * recip[t - 1]
        tmp = we[t - 1].T @ c
        f = tmp.max()
        g[t] = 1.0 / f
        Cacc += np.log(f)
        beta = tmp * g[t]

    rsg = np.ones((W, L), dtype=f64)
    rsg[:, 1:] = recip.T * g[None, 1:]
    Lsc = (Ll[:, None, :] * rsg[:, :, None]).astype(np.float32)  # (W,L,NB)

    wt = np.ascontiguousarray(
        we.transpose(1, 0, 2)).astype(ml_dtypes.bfloat16)  # (W,255,W)
    ones_v = np.ones((W, 1), dtype=ml_dtypes.bfloat16)
    return wt, Lsc, bins, ones_v, Cacc


def _build_em_pair(Lsc, bins_core, pi):
    """em stream for pair pi: [W, steps, 2*BC] bf16.
    em[:, i, h*BC:(h+1)*BC] = Lsc[:, t, bins[:, t]] for chain h at step i."""
    steps = PAIR_STEPS[pi]
    out = np.empty((W, steps, 2 * BC), dtype=ml_dtypes.bfloat16)
    for h, s in enumerate(PAIRS[pi]):
        ts = CHAIN_THI[s] - np.arange(steps)                # (steps,)
        # gather: for each step i: Lsc[:, ts[i], bins_core[:, ts[i]]]
        bsel = bins_core[:, ts]                             # (BC, steps)
        blk = Lsc[:, ts[:, None], bsel.T]                   # (W, steps, BC)
        out[:, :, h * BC:(h + 1) * BC] = blk.astype(ml_dtypes.bfloat16)
    return out


def kernel(data, input_distros, dense_layer_weights):
    global LAST_LNY, LAST_RESULTS, _CACHED
    wt, Lsc, bins, ones_v, Cacc = _host_prep(
        np.asarray(data), np.asarray(input_distros),
        np.asarray(dense_layer_weights))

    if _CACHED is None:
        _CACHED = _build_nc()
    nc = _CACHED

    in_maps = []
    for cid in range(NCORES):
        bins_core = bins[cid * BC:(cid + 1) * BC, :]        # (BC, L)
        m = {"wt": wt, "ones": ones_v}
        for pi in range(3):
            m[f"em{pi}"] = _build_em_pair(Lsc, bins_core, pi)
        in_maps.append(m)

    res = run_bass_kernel_spmd(
        nc, in_maps, core_ids=list(range(NCORES)),
        trace=bool(int(os.environ.get("KERNEL_TRACE", "0"))))
    LAST_RESULTS = res

    # rows: emitted in program order per junction_sum() calls:
    #  pair0 chains (1,2) then pair1 (3,4) then pair2 (5,6), interleaved by
    #  step; mapping below reconstructs which row is which junction sum.
    rows = {}
    ri = 0
    for i in range(max(PAIR_STEPS)):
        for pi, (sa, sb_id) in enumerate(PAIRS):
            if i >= PAIR_STEPS[pi]:
                continue
            for s in (sa, sb_id):
                t = CHAIN_THI[s] - i
                if t == CHAIN_TLO[s]:
                    rows[("lo", s)] = ri; ri += 1
                elif t == M_BOUND[s] and s < 6:
                    rows[("burn", s)] = ri; ri += 1
    assert ri == 11, ri

    lnY = np.zeros(B, dtype=np.float64)
    for cid in range(NCORES):
        jr = LAST_RESULTS.results[cid]["juncs"].astype(np.float64)
        acc = np.log(jr[rows[("lo", 1)]])                   # colsum at t=0
        for s in range(1, 6):
            acc += np.log(jr[rows[("lo", s + 1)]])          # J_upper
            acc -= np.log(jr[rows[("burn", s)]])            # J_burn-in
        lnY[cid * BC:(cid + 1) * BC] = acc + Cacc
    LAST_LNY = lnY
    y = np.exp(lnY).astype(np.float32).reshape(B, 1)
    return y
